# revision 1
# baseline (speedup 1.0000x reference)
"""Trainium2 Bass kernel for nn_MultiHeadAttentionBlock (B=2, L=2048, D=1024, H=16).

Sharding: 8 cores = 2 batches x 4 head-groups (4 heads each), Megatron-style.
Each core computes q/k/v projections for its 4 heads (column-sharded weights),
RoPE, attention, and a partial output projection (row-sharded w_o). The host
sums the 4 partial outputs per batch (the "all-reduce").

v2: fully software-pipelined single schedule.
  - DMA order: wk+kT first, then wq+qT, tables+mask(t1=0), wv+vT, wo.
  - The attention phase keeps the Activation engine exp-only (exp of the
    full P matrix is the per-core floor at ~144us); all PSUM evacuations
    run on Pool/DVE.
  - attn@V runs "flipped": lhsT = P chunks [128k,128q], rhs = [V|1] so the
    PSUM accumulators are token-major [128q, 65] at full partition use --
    half the PE columns of the feature-major variant -- and softmax
    normalization becomes a per-partition reciprocal+scale (no partition
    broadcast). A PE transpose returns O to feature-major for w_o.
  - V projection is interleaved into attention blocks 0-1; the w_o
    projection of t1 is interleaved into blocks 2t1+2..3; outputs stream
    to DRAM per 128-token tile.
  - PSUM budget (8 banks): scores 2x[128,1024] (4) + acc 2x[128,512] (2)
    + aux ring 2x[128,512] f32 (2) shared by V-proj psums, w_o psums and
    the O-transpose target.
"""

import contextlib
import sys

import numpy as np

sys.path.insert(0, "/opt/trn_rl_repo")

import ml_dtypes  # noqa: E402

import concourse.bass as bass  # noqa: E402
import concourse.tile as tile  # noqa: E402
from concourse import bacc, mybir  # noqa: E402
from concourse.bass import ts  # noqa: E402

F32 = mybir.dt.float32
BF16 = mybir.dt.bfloat16
FP16 = mybir.dt.float16
AF = mybir.ActivationFunctionType

B, L, D, H = 2, 2048, 1024, 16
DK = D // H          # 64
HG = 4               # heads per core
DH = HG * DK         # 256 features per core
N_CORES = 8
KC = D // 128        # 8 contraction chunks for projections
T1C = 4              # number of 512-wide query chunks
T2C = 16             # number of 128-wide key chunks


def build_kernel():
    nc = bacc.Bacc(
        "TRN2",
        target_bir_lowering=False,
        debug=False,
        enable_asserts=False,
        num_devices=N_CORES,
    )

    qT = nc.dram_tensor("qT", [D, L], FP16, kind="ExternalInput").ap()
    kT = nc.dram_tensor("kT", [D, L], FP16, kind="ExternalInput").ap()
    vT = nc.dram_tensor("vT", [D, L], FP16, kind="ExternalInput").ap()
    wq = nc.dram_tensor("wq", [D, DH], FP16, kind="ExternalInput").ap()
    wk = nc.dram_tensor("wk", [D, DH], FP16, kind="ExternalInput").ap()
    wv = nc.dram_tensor("wv", [D, DH], FP16, kind="ExternalInput").ap()
    wo = nc.dram_tensor("wo", [DH, D], FP16, kind="ExternalInput").ap()
    cosT = nc.dram_tensor("cosT", [128, L], FP16, kind="ExternalInput").ap()
    sinT = nc.dram_tensor("sinT", [128, L], FP16, kind="ExternalInput").ap()
    maskT = nc.dram_tensor("maskT", [L, L], FP16, kind="ExternalInput").ap()
    ident = nc.dram_tensor("ident", [128, 128], F32, kind="ExternalInput").ap()
    out = nc.dram_tensor("out", [L, D], FP16, kind="ExternalOutput").ap()

    qT_c = qT.rearrange("(c p) n -> p c n", p=128)        # [128, 8, 2048]
    kT_c = kT.rearrange("(c p) n -> p c n", p=128)
    vT_c = vT.rearrange("(c p) n -> p c n", p=128)
    wq_c = wq.rearrange("(c p) n -> p c n", p=128)        # [128, 8, 256]
    wk_c = wk.rearrange("(c p) n -> p c n", p=128)
    wv_c = wv.rearrange("(c p) n -> p c n", p=128)
    wo_c = wo.rearrange("(c p) n -> p c n", p=128)        # [128, 2, 1024]
    maskT_c = maskT.rearrange("(c p) n -> p c n", p=128)  # [128, 16, 2048]
    out_c = out.rearrange("(t p) n -> p t n", p=128)      # [128, 16, 1024]

    with tile.TileContext(nc) as tc, contextlib.ExitStack() as top:
        persist = top.enter_context(tc.tile_pool(name="persist", bufs=1))
        mpool = top.enter_context(tc.tile_pool(name="mask", bufs=3))
        pmpool = top.enter_context(tc.tile_pool(name="pm", bufs=7))
        pepool = top.enter_context(tc.tile_pool(name="pex", bufs=3))
        oqpool = top.enter_context(tc.tile_pool(name="oq", bufs=4))
        ospool = top.enter_context(tc.tile_pool(name="ostage", bufs=3))
        smallp = top.enter_context(tc.tile_pool(name="small", bufs=4))
        vxpool = top.enter_context(tc.tile_pool(name="vx", bufs=1))

        # ---- persistent SBUF ------------------------------------------------
        KT_hc = [persist.tile([128, L], FP16, name=f"KThc{p}", tag=f"KThc{p}")
                 for p in range(2)]
        QT_hc = [persist.tile([128, L], FP16, name=f"QThc{p}", tag=f"QThc{p}")
                 for p in range(2)]
        V_all = persist.tile([128, T2C, HG, DK + 1], FP16, name="V_all",
                             tag="V_all")
        OT_sb = [persist.tile([128, L], FP16, name=f"OTsb{p}", tag=f"OT{p}")
                 for p in range(2)]
        wk_sb = persist.tile([128, KC, DH], FP16, name="wk_sb", tag="wk")
        wq_sb = persist.tile([128, KC, DH], FP16, name="wq_sb", tag="wq")
        wv_sb = persist.tile([128, KC, DH], FP16, name="wv_sb", tag="wv")
        wo_sb = persist.tile([128, 2, D], FP16, name="wo_sb", tag="wo")
        cos_h = persist.tile([128, L], FP16, name="cos_h", tag="cos")
        sin_h = persist.tile([128, L], FP16, name="sin_h", tag="sin")
        id_sb = persist.tile([128, 128], F32, name="id_sb", tag="ident")

        mt_tiles = {}

        # ---- phase B + C under shared transient scopes ----------------------
        NKX = 3  # kT streaming ring
        with tc.tile_pool(name="xs", bufs=1) as xspool, \
             tc.tile_pool(name="qx", bufs=1) as qxpool, \
             tc.tile_pool(name="pp", bufs=1, space="PSUM") as pp, \
             tc.tile_pool(name="rt", bufs=1) as rt, \
             tc.tile_pool(name="preroped", bufs=1) as prp:

            # ---- DMA preamble (order = priority; everything queued here
            # stripes across the 16 DMA engines concurrently, so the queue
            # order below decides which stream finishes first) --------------
            nc.sync.dma_start(wk_sb[:], wk_c)
            nc.sync.dma_start(wq_sb[:], wq_c)
            # kT streams through a ring of NKX buffers; chunks >= NKX are
            # DMA'd from inside the projection loop (after the matmuls that
            # free their ring slot have been emitted)
            def dma_chunk2(xt, view):
                nc.sync.dma_start(xt[:], view)

            kxt = []
            for kk in range(NKX):
                xt = xspool.tile([128, L], FP16, name=f"kx{kk}",
                                 tag=f"x{kk % NKX}")
                dma_chunk2(xt, kT_c[:, kk, :])
                kxt.append(xt)
            qxt = []
            for kk in range(KC):
                xt = qxpool.tile([128, L], FP16, name=f"qx{kk}", tag=f"q{kk}")
                dma_chunk2(xt, qT_c[:, kk, :])
                qxt.append(xt)
            nc.vector.memset(V_all[:, :, :, DK:DK + 1], 1.0)
            nc.sync.dma_start(cos_h[:], cosT)
            nc.sync.dma_start(sin_h[:], sinT)

            def late_dmas():
                # queued after the K-proj emission so the kT ring prefetches
                # stripe ahead of this ~7MB of later-needed input
                for hf in range(2):
                    mt_tiles[(0, hf)] = mpool.tile([128, 8, 512], FP16,
                                                   name=f"mt0{hf}", tag="mask")
                    nc.sync.dma_start(mt_tiles[(0, hf)][:],
                                      maskT_c[:, ts(hf, 8), ts(0, 512)])
                nc.sync.dma_start(wv_sb[:], wv_c)
                for kk in range(KC):
                    xt = vxpool.tile([128, L], FP16, name=f"vx{kk}",
                                     tag=f"v{kk}")
                    nc.sync.dma_start(xt[:], vT_c[:, kk, :])
                    vxt.append(xt)
                nc.sync.dma_start(wo_sb[:], wo_c)
                nc.sync.dma_start(id_sb[:], ident)

            vxt = []

            # ---- phase B: projections + rope + repack -----------------------
            def proj_rope(xtiles, w_sb, dst0, dst1, prefetch=None):
                ps = [pp.tile([128, 1024], F32, name=f"ps{q}", tag=f"ps{q}")
                      for q in range(4)]  # index fh*2+th
                for kk in range(KC):
                    for fh in range(2):
                        for th in range(2):
                            p_ = ps[fh * 2 + th]
                            for n in range(2):
                                nc.tensor.matmul(
                                    p_[:, ts(n, 512)],
                                    lhsT=w_sb[:, kk, ts(fh, 128)],
                                    rhs=xtiles[kk][:, th * 1024 + n * 512:
                                                   th * 1024 + (n + 1) * 512],
                                    start=(kk == 0),
                                    stop=(kk == KC - 1),
                                )
                    if prefetch is not None:
                        prefetch(kk)
                for th in range(2):
                    rope_th(ps[th], ps[2 + th], th, dst0, dst1)

            def rope_th(ps0, ps1, th, dst0, dst1, act_copy=True):
                # rope: dst0 = x0*c - x1*s ; dst1 = x1*c + x0*s.
                # act_copy=False routes the PSUM evacuations to DVE so the
                # ACT engine's in-order stream can reach the first exp of
                # phase C without waiting on this rope.
                x0f = rt.tile([128, 1024], FP16, name="x0f", tag="x0f")
                x1f = rt.tile([128, 1024], FP16, name="x1f", tag="x1f")
                if act_copy:
                    nc.scalar.copy(x0f[:], ps0[:])
                    nc.scalar.copy(x1f[:], ps1[:])
                else:
                    nc.vector.tensor_copy(x0f[:], ps0[:])
                    nc.vector.tensor_copy(x1f[:], ps1[:])
                c = cos_h[:, ts(th, 1024)]
                s = sin_h[:, ts(th, 1024)]
                x0c = rt.tile([128, 1024], FP16, name="x0c", tag="x0c")
                x1s = rt.tile([128, 1024], FP16, name="x1s", tag="x1s")
                x1c = rt.tile([128, 1024], FP16, name="x1c", tag="x1c")
                x0s = rt.tile([128, 1024], FP16, name="x0s", tag="x0s")
                nc.vector.tensor_mul(x0c[:], x0f[:], c)
                nc.vector.tensor_mul(x1s[:], x1f[:], s)
                nc.vector.tensor_mul(x1c[:], x1f[:], c)
                nc.vector.tensor_mul(x0s[:], x0f[:], s)
                nc.vector.tensor_sub(dst0[:, ts(th, 1024)], x0c[:], x1s[:])
                nc.vector.tensor_add(dst1[:, ts(th, 1024)], x1c[:], x0s[:])

            def proj_rope_staged(xtiles, w_sb, dst0, dst1, dst_hc):
                # th-staged: token-half th's rope + repack run on ACT/DVE
                # while the other half's contraction sweep runs on the PE
                for th in range(2):
                    ps = [pp.tile([128, 1024], F32, name=f"ps{fh}",
                                  tag=f"ps{fh * 2 + th}") for fh in range(2)]
                    for kk in range(KC):
                        for fh in range(2):
                            for n in range(2):
                                nc.tensor.matmul(
                                    ps[fh][:, ts(n, 512)],
                                    lhsT=w_sb[:, kk, ts(fh, 128)],
                                    rhs=xtiles[kk][:, th * 1024 + n * 512:
                                                   th * 1024 + (n + 1) * 512],
                                    start=(kk == 0),
                                    stop=(kk == KC - 1),
                                )
                    rope_th(ps[0], ps[1], th, dst0, dst1)
                    repack(dst0, dst1, dst_hc, th)

            def repack(src0, src1, dst, th=None):
                # head-contiguous: dst[p][64j+32*half+..] <- src[half][32hh+..]
                sl = slice(0, L) if th is None else slice(th * 1024,
                                                          (th + 1) * 1024)
                for hh in range(HG):
                    p_, j_ = divmod(hh, 2)
                    for half, src in enumerate((src0, src1)):
                        nc.vector.tensor_copy(
                            dst[p_][64 * j_ + 32 * half:
                                    64 * j_ + 32 * half + 32, sl],
                            src[32 * hh:32 * hh + 32, sl])

            def k_prefetch(kk):
                nk = kk + NKX
                if nk < KC:
                    xt = xspool.tile([128, L], FP16, name=f"kx{nk}",
                                     tag=f"x{nk % NKX}")
                    dma_chunk2(xt, kT_c[:, nk, :])
                    kxt.append(xt)

            KT_sb0 = prp.tile([128, L], FP16, name="KTsb0", tag="pr0")
            KT_sb1 = prp.tile([128, L], FP16, name="KTsb1", tag="pr1")
            proj_rope(kxt, wk_sb, KT_sb0, KT_sb1, prefetch=k_prefetch)
            late_dmas()
            repack(KT_sb0, KT_sb1, KT_hc)

            QT_sb0 = prp.tile([128, L], FP16, name="QTsb0", tag="pr0")
            QT_sb1 = prp.tile([128, L], FP16, name="QTsb1", tag="pr1")
            proj_rope_staged(qxt, wq_sb, QT_sb0, QT_sb1, QT_hc)

        # ---- phase C: pipelined attention blocks ----------------------------
        # block b = t1*2 + p; steps i = 0..15 per block:
        #   steps 0,1: tail of previous block's attn@V
        #   step 2: normalize(prev block) (DVE); step 3: PE transpose + evac
        #   steps lag..15: this block's attn@V, i2 = i - lag
        #   every step: scores(b,i) -> exp (ACT) -> mask-mul (DVE) -> pm ring
        #   blocks 0-1: V-projection interleave; blocks >=2: w_o projection
        with tc.tile_pool(name="att_psum", bufs=1, space="PSUM") as apsum, \
             tc.tile_pool(name="aux_psum", bufs=2, space="PSUM") as aux:

            def scores_mm(b, i):
                t1, p = b // 2, b % 2
                psc = apsum.tile([128, 1024], F32, name="psc", tag="psc",
                                 bufs=2)
                for j in range(2):
                    nc.tensor.matmul(
                        psc[:, ts(j, 512)],
                        lhsT=KT_hc[p][ts(j, 64), ts(i, 128)],
                        rhs=QT_hc[p][ts(j, 64), ts(t1, 512)],
                        start=True, stop=True,
                        tile_position=(64 * j, 0),
                    )
                return psc

            def exp_mask(b, i, psc):
                t1 = b // 2
                pex = pepool.tile([128, 1024], FP16, name="pex", tag="pex")
                nc.scalar.activation(pex[:], psc[:], AF.Exp)
                pm = pmpool.tile([128, 1024], FP16, name="pm", tag="pm")
                # every 4th mask-multiply runs on Pool (SBUF-only there) to
                # keep DVE below the ACT exp ceiling
                eng = nc.gpsimd if i % 4 == 3 else nc.vector
                eng.tensor_mul(
                    pm[:], pex[:],
                    mt_tiles[(t1, i // 8)][:, i % 8, None, :]
                    .broadcast_to([128, 2, 512]))
                return pm

            def attnv_mm(b, i2, pm, accs):
                # one psum accumulation group per acc bank: the 4 qc
                # sub-tiles share the bank's 2KB zero region
                p = b % 2
                for j in range(2):
                    for qc in range(4):
                        nc.tensor.matmul(
                            accs[j][:, qc * 128:qc * 128 + DK + 1],
                            lhsT=pm[:, j * 512 + qc * 128:
                                    j * 512 + (qc + 1) * 128],
                            rhs=V_all[:, i2, 2 * p + j, :],
                            start=(i2 == 0 and qc == 0),
                            stop=(i2 == T2C - 1 and qc == 3),
                        )

            def vproj(tt):
                pv = aux.tile([128, 512], F32, name="pv", tag="aux")
                for kk in range(KC):
                    nc.tensor.matmul(
                        pv[:, 0:DH],
                        lhsT=vxt[kk][:, ts(tt, 128)],
                        rhs=wv_sb[:, kk, :],
                        start=(kk == 0),
                        stop=(kk == KC - 1),
                    )
                nc.vector.tensor_copy(
                    V_all[:, tt, :, 0:DK],
                    pv[:, 0:DH].rearrange("p (h d) -> p h d", h=HG))

            def normalize(b, accs):
                # accs[j][:, qc*128 : qc*128+64] numerators (token-major),
                # col qc*128+64 the softmax denominator; scale by 1/denom
                # (per-partition scalar) into oq tiles [128q, 128dh].
                oqs = []
                for qc in range(4):
                    oq = oqpool.tile([128, 128], F32, name="oq", tag="oq")
                    for j in range(2):
                        rc = smallp.tile([128, 1], F32, name="rc", tag="rc")
                        nc.vector.reciprocal_approx_fast(
                            rc[:], accs[j][:, qc * 128 + DK:qc * 128 + DK + 1])
                        nc.vector.tensor_scalar_mul(
                            oq[:, ts(j, DK)],
                            accs[j][:, qc * 128:qc * 128 + DK], rc[:])
                    oqs.append(oq)
                return oqs

            def transpose_evac(b, oqs):
                t1, p = b // 2, b % 2
                tp = aux.tile([128, 512], F32, name="tp", tag="aux")
                for qc in range(4):
                    nc.tensor.matmul(
                        tp[:, ts(qc, 128)], lhsT=oqs[qc][:], rhs=id_sb[:],
                        is_transpose=True, start=True, stop=True,
                    )
                nc.vector.tensor_copy(OT_sb[p][:, ts(t1, 512)], tp[:])

            def outproj_chunk(t1o, ci, psc_bank=False):
                # ci in 0..7: t-tile = 4*t1o + ci//2, column half ci%2.
                # psc_bank (tail only): borrow a scores psum buffer -- the
                # scores ring is idle after the last block, and 4 buffers
                # in rotation double the tail chain's concurrency.
                t = 4 * t1o + ci // 2
                jj = ci % 2
                if psc_bank:
                    po = apsum.tile([128, 1024], F32, name="po2", tag="psc",
                                    bufs=2)[:, 0:512]
                else:
                    po = aux.tile([128, 512], F32, name="po", tag="aux")[:]
                for pp_ in range(2):
                    nc.tensor.matmul(
                        po,
                        lhsT=OT_sb[pp_][:, ts(t, 128)],
                        rhs=wo_sb[:, pp_, ts(jj, 512)],
                        start=(pp_ == 0),
                        stop=(pp_ == 1),
                    )
                ob = ospool.tile([128, 512], FP16, name="ob", tag="ob")
                nc.vector.tensor_copy(ob[:], po)
                nc.sync.dma_start(out_c[:, t, ts(jj, 512)], ob[:])

            pm_hist = {}      # (b, i) -> pm tile
            acc_hist = {}     # b -> accs
            oq_hist = {}      # b -> oq tiles
            # attn@V runs 4 steps behind its pm tile; leftovers {12..15}
            # finish in the next block's first two steps
            TAIL_NEW = ((12, 13), (14, 15))
            TAIL_B0 = ((12, 13), (14, 15))
            for b in range(8):
                t1, p = b // 2, b % 2
                accs = [apsum.tile([128, 512], F32, name=f"acc{j}",
                                   tag=f"acc{j}") for j in range(2)]
                acc_hist[b] = accs
                for i in range(T2C):
                    # mask prefetch (half-granular, ring of 3): (t1+1, 0) at
                    # odd-block step 0, (t1+1, 1) at odd-block step 8 --
                    # each lands after the half whose buffer it reuses dies
                    if p == 1 and t1 + 1 < T1C and i in (0, 8):
                        hf = i // 8
                        mt_tiles[(t1 + 1, hf)] = mpool.tile(
                            [128, 8, 512], FP16, name=f"mt{t1+1}{hf}",
                            tag="mask")
                        nc.sync.dma_start(mt_tiles[(t1 + 1, hf)][:],
                                          maskT_c[:, ts(hf, 8),
                                                  ts(t1 + 1, 512)])
                    # previous block's attn@V tail, spread over steps 0-1
                    if b >= 1 and i < 2:
                        pb = b - 1
                        tail = TAIL_B0 if pb == 0 else TAIL_NEW
                        for i2 in tail[i]:
                            attnv_mm(pb, i2, pm_hist[(pb, i2)], acc_hist[pb])
                    if b >= 1 and i == 2:
                        oq_hist[b - 1] = normalize(b - 1, acc_hist[b - 1])
                        del acc_hist[b - 1]
                    if b >= 1 and i == 3:
                        transpose_evac(b - 1, oq_hist.pop(b - 1))
                    # this block's attn@V (lag 4)
                    if i >= 4:
                        i2 = i - 4
                        attnv_mm(b, i2, pm_hist[(b, i2)], accs)
                    # scores -> exp -> mask
                    psc = scores_mm(b, i)
                    pm_hist[(b, i)] = exp_mask(b, i, psc)
                    # V projection interleave (block 0: tiles 0-12 at steps
                    # 3-15, tiles 13-15 doubled up on the last steps so the
                    # block-0 attn@V tail in block 1 finds them ready)
                    if b == 0 and 3 <= i:
                        vproj(i - 3)
                        if i >= 13:
                            vproj(i)
                    # w_o projection interleave (blocks >= 2)
                    if b >= 2:
                        t1o = (b - 2) // 2
                        if b % 2 == 0 and i in (8, 10, 12, 14):
                            outproj_chunk(t1o, (i - 8) // 2)
                        elif b % 2 == 1 and i in (2, 4, 6, 8):
                            outproj_chunk(t1o, 4 + (i - 2) // 2)

            # ---- tail: finish block 7, then t1=3 output projection ----------
            b = 7
            for i2 in TAIL_NEW[0] + TAIL_NEW[1]:
                attnv_mm(b, i2, pm_hist[(b, i2)], acc_hist[b])
            oqs = normalize(b, acc_hist[b])
            transpose_evac(b, oqs)
            for ci in range(8):
                outproj_chunk(3, ci, psc_bank=(ci % 2 == 1))

    nc.compile()
    return nc


def shard_inputs(q, k, v, mask, w_q, w_k, w_v, w_o):
    q = np.asarray(q, np.float32)
    k = np.asarray(k, np.float32)
    v = np.asarray(v, np.float32)
    w_q = np.asarray(w_q, np.float32)
    w_k = np.asarray(w_k, np.float32)
    w_v = np.asarray(w_v, np.float32)
    w_o = np.asarray(w_o, np.float32)
    mask = np.asarray(mask)

    qT = [np.ascontiguousarray(q[b].T).astype(np.float16) for b in range(B)]
    kT = [np.ascontiguousarray(k[b].T).astype(np.float16) for b in range(B)]
    vT = [np.ascontiguousarray(v[b].T).astype(np.float16) for b in range(B)]
    maskT_bf = np.ascontiguousarray(mask[0, 0].T).astype(np.float16)

    inv = 1.0 / (10000.0 ** (np.arange(0, DK, 2) / DK))   # [32]
    t = np.arange(L)
    fr = np.outer(inv, t)                                 # [32, 2048]
    cos_tab = np.tile(np.cos(fr), (4, 1)).astype(np.float16)  # [128, 2048]
    sin_tab = np.tile(np.sin(fr), (4, 1)).astype(np.float16)
    ident = np.eye(128, dtype=np.float32)

    even = np.arange(0, DK, 2)
    odd = np.arange(1, DK, 2)
    scale = 1.0 / np.sqrt(DK)

    in_maps = []
    for core in range(N_CORES):
        b, g = divmod(core, N_CORES // B)
        hs = [HG * g + i for i in range(HG)]
        rows_qk = np.concatenate([h * DK + even for h in hs]
                                 + [h * DK + odd for h in hs])
        rows_v = np.concatenate([np.arange(h * DK, (h + 1) * DK) for h in hs])
        in_maps.append({
            "qT": qT[b],
            "kT": kT[b],
            "vT": vT[b],
            "wq": np.ascontiguousarray((w_q[rows_qk, :] * scale).T).astype(np.float16),
            "wk": np.ascontiguousarray(w_k[rows_qk, :].T).astype(np.float16),
            "wv": np.ascontiguousarray(w_v[rows_v, :].T).astype(np.float16),
            "wo": np.ascontiguousarray(w_o[:, rows_v].T).astype(np.float16),
            "cosT": cos_tab,
            "sinT": sin_tab,
            "maskT": maskT_bf,
            "ident": ident,
        })
    return in_maps


_compiled = None


def _get_compiled():
    global _compiled
    if _compiled is None:
        _compiled = build_kernel()
    return _compiled


def kernel(q, k, v, mask, w_q, w_k, w_v, w_o, _trace=False, _trace_cores=None):
    from concourse.bass_utils import run_bass_kernel_spmd

    nc = _get_compiled()
    in_maps = shard_inputs(q, k, v, mask, w_q, w_k, w_v, w_o)
    res = run_bass_kernel_spmd(
        nc, in_maps, core_ids=list(range(N_CORES)),
        trace=_trace, trace_cores=_trace_cores,
    )
    out = np.zeros((B, L, D), np.float32)
    for core in range(N_CORES):
        out[core // (N_CORES // B)] += res.results[core]["out"].astype(np.float32)
    kernel._last_results = res
    return out

